# revision 8
# baseline (speedup 1.0000x reference)
"""Causal multi-head attention on 8 trn2 NeuronCores.

Problem: B=2, S=2048, D=1024, H=16 heads, HD=64. fp32 in/out.

Sharding: 8 cores = 2 (batch) x 4 (head groups of 4 heads).
Each core computes, for its batch b and head group g:
  Q^T,K^T [256, 2048] (dg on partitions, seq on free) = W^T-slice @ x
  V       [2048, 4*(64+1)]  (natural, a ones column per head)
  per 512-wide q chunk, per head-pair: for each k tile j:
    S^T[k,q] both heads as a ROW-TILED CONCURRENT matmul pair (K=64 each,
    tile_position rows 0-63 / 64-127, ~1.9x PE throughput measured) into
    one [128, 1024] PSUM tile;
    P = exp(S^T/8 - 4) on ACT (fp16 out; the -4 offset prevents fp16
    overflow and cancels exactly in the softmax ratio);
    causal: diagonal k-tiles narrowed to valid q columns, plus a [128,128]
    triangle mask on the diagonal block (split DVE/gpsimd, one per head,
    so neither strict-FIFO queue delays PV);
    PV accumulated over j with V_aug stationary (m=65; row 64 = softmax
    denominator), software-pipelined 4 j-steps behind QK so the ACT exp
    and mask latency never stall the PE FIFO.
  Normalize (deferred 2 ticks into the next pair's loop): DVE reciprocal
  of the psum denominator row, gpsimd partition_broadcast, DVE multiply
  psum->ctx^T (fp16).
  O_partial = ctx^T.T @ Wo_rows [2048, 1024] (psum->sbuf copies split
  DVE/gpsimd, output DMA on the SP HWDGE queue).
Emission interleaves next-chunk projections (and trailing Wo tiles) into
the attention j-loops so projection copy work never bunches up at chunk
seams. Engine balance: PE matmuls; ACT exp only; DVE qk copies +
masks(h0) + normalize + half Wo copies; gpsimd V copies + masks(h1) +
broadcast + half Wo copies.
Host: sums the 4 head-group partials per batch and adds bo + bv @ Wo.
bq/bk are structurally zero in this problem; a biased build variant
(ACT bias-add, as in v1) is selected at runtime if they are not.

Input DMA: 12 configs total (weights as single [128, k, m] tiles, xt as
8 full rows); wv+wq ride the ACT HWDGE queue so SP config serialization
never delays the first projections. A short dummy-matmul warmup stream
keeps the PE p-state ramp off the critical path of a cold exec.

All matmul operands fp16 (1 cycle/row PE rate, halved DMA + SBUF);
accumulation is always fp32 PSUM.
"""

import sys

if "/opt/trn_rl_repo" not in sys.path:
    sys.path.insert(0, "/opt/trn_rl_repo")

import numpy as np

import concourse.bacc as bacc
import concourse.bass as bass
import concourse.mybir as mybir
import concourse.tile as tile
from concourse.bass_utils import run_bass_kernel_spmd

B, S, D, H = 2, 2048, 1024, 16
HD = D // H  # 64
N_CORES = 8
HEADS_PER_CORE = H // 4  # 4
DG = HEADS_PER_CORE * HD  # 256 head dims per core
P = 128
CHUNK = 512  # q chunk width
N_KT = S // P  # 16 k tiles
N_CH = S // CHUNK  # 4 q chunks
F32 = mybir.dt.float32
F16 = mybir.dt.float16
EXP_BIAS = -4.0  # exp(s/8 - 4): fp16-overflow guard, cancels in softmax

_CACHE = {}


def build_kernel(mm_dt="f16in", unroll=1, ablate=(), biased=False,
                 warmup=True):
    """Build + compile the per-core SPMD program. unroll>1 wraps the body
    in a hardware loop (for pure device timing measurements)."""
    nc = bacc.Bacc("TRN2", target_bir_lowering=False, debug=False)
    xT_d = nc.dram_tensor("xT", [D, S], F16, kind="ExternalInput")
    wq_d = nc.dram_tensor("wq", [D, DG], F16, kind="ExternalInput")
    wk_d = nc.dram_tensor("wk", [D, DG], F16, kind="ExternalInput")
    wv_d = nc.dram_tensor("wv", [D, DG], F16, kind="ExternalInput")
    wo_d = nc.dram_tensor("wo", [DG, D], F16, kind="ExternalInput")
    if biased:
        bq_d = nc.dram_tensor("bq", [DG, 1], F32, kind="ExternalInput")
        bk_d = nc.dram_tensor("bk", [DG, 1], F32, kind="ExternalInput")
    else:
        bq_d = bk_d = None
    o_d = nc.dram_tensor("o", [S, D], F16, kind="ExternalOutput")

    NDT = D // P  # 8 contraction tiles over D
    NMT = DG // P  # 2 m-tiles over the core's head dims (= head pairs)

    with tile.TileContext(nc) as tc:
        _body(tc, nc,
              xT_d, wq_d, wk_d, wv_d, wo_d, bq_d, bk_d, o_d, NDT, NMT,
              ablate, unroll, warmup)

    nc.compile()
    return nc


def _body(tc, nc, xT_d, wq_d, wk_d, wv_d, wo_d, bq_d, bk_d, o_d,
          NDT, NMT, ablate=(), unroll=1, warmup=True):
    import contextlib
    ctx = contextlib.ExitStack()
    biased = bq_d is not None
    with ctx:
        const = ctx.enter_context(tc.tile_pool(name="const", bufs=1))
        sbuf = ctx.enter_context(tc.tile_pool(name="sbuf", bufs=1))
        ptile_p = ctx.enter_context(tc.tile_pool(name="ptile", bufs=8))
        den_p = ctx.enter_context(tc.tile_pool(name="den", bufs=6))
        out_p = ctx.enter_context(tc.tile_pool(name="outp", bufs=3))
        qkv_ps = ctx.enter_context(
            tc.tile_pool(name="qkv_ps", bufs=2, space="PSUM"))
        stp_ps = ctx.enter_context(
            tc.tile_pool(name="stp_ps", bufs=2, space="PSUM"))
        pv_ps = ctx.enter_context(
            tc.tile_pool(name="pv_ps", bufs=2, space="PSUM"))

        # ---- input tiles ------------------------------------------------
        xt = [const.tile([P, S], F16, tag=f"xt{i}", name=f"xt{i}")
              for i in range(NDT)]
        # weights as single tiles [128, k-tile, dg] -> one DMA config each
        ws = {name: const.tile([P, NDT, DG], F16, tag=name, name=name)
              for name in ("wq", "wk", "wv")}
        wo = const.tile([P, NMT, D], F16, tag="wo", name="wo")
        if biased:
            biases = {(name, m): const.tile([P, 1], F32, tag=f"{name}{m}",
                                            name=f"{name}{m}")
                      for name in ("bq", "bk") for m in range(NMT)}

        def emit_in_dma():
            # All inputs on the SP queue, earliest-needed first. Outputs ride
            # the gpsimd SWDGE queue, so neither HWDGE queue ever holds an
            # iteration's input prefetch behind the previous iteration's
            # late output tiles.
            nc.sync.dma_start(
                ws["wv"][:],
                wv_d.ap().rearrange("(k p) d -> p k d", p=P))
            for k in range(NDT):
                nc.sync.dma_start(xt[k][:], xT_d.ap()[P * k:P * (k + 1), :])
            nc.sync.dma_start(
                ws["wq"][:],
                wq_d.ap().rearrange("(k p) d -> p k d", p=P))
            nc.sync.dma_start(
                ws["wk"][:],
                wk_d.ap().rearrange("(k p) d -> p k d", p=P))
            nc.sync.dma_start(
                wo[:], wo_d.ap().rearrange("(m p) d -> p m d", p=P))
            if biased:
                for (name, m), t in biases.items():
                    d = bq_d if name == "bq" else bk_d
                    nc.sync.dma_start(t[:], d.ap()[P * m:P * (m + 1), :])

        # ---- constants: vaug ones + causal triangle mask ----------------
        ones_f = const.tile([P, HEADS_PER_CORE], F32, tag="ones_f",
                            name="ones_f")
        ones_r = const.tile([P, HEADS_PER_CORE], F16, tag="ones_r",
                            name="ones_r")
        ebias = const.tile([P, 1], F32, tag="ebias", name="ebias")
        m01 = const.tile([P, P], F16, tag="m01", name="m01")
        wup = const.tile([P, CHUNK], F16, tag="wup", name="wup")

        def emit_consts():
            nc.vector.memset(ones_f[:], 1.0)
            nc.vector.tensor_copy(ones_r[:], ones_f[:])
            nc.vector.memset(ebias[:], EXP_BIAS)
            # m01[r, c] = 1 if c >= r else 0 (causal triangle, q >= key)
            nc.gpsimd.memset(m01[:], 1.0)
            nc.gpsimd.affine_select(
                out=m01[:], in_=m01[:],
                compare_op=mybir.AluOpType.is_ge,
                fill=0.0, base=0, pattern=[[1, P]],
                channel_multiplier=-1)

        def emit_warmup(n=24):
            # dummy matmuls with no DMA dependency: keep the PE busy from
            # t=0 so the p-state ramp completes during the input-DMA
            # prologue instead of on the first real matmuls
            nc.vector.memset(wup[:], 0.0)
            ps = qkv_ps.tile([P, CHUNK], F32, tag="proj", name="proj")
            for i in range(n):
                nc.tensor.matmul(ps[:], wup[:, 0:P], wup[:],
                                 start=True, stop=True)

        # ---- V projection (natural layout + ones cols) ------------------
        # vaug[j]: [128, 4*65]; head h cols h*65..h*65+63 = V, col h*65+64 = 1
        vaug = [sbuf.tile([P, HEADS_PER_CORE * (HD + 1)], F16,
                          tag=f"vaug{j}", name=f"vaug{j}")
                for j in range(N_KT)]

        def v_proj(j):
            ps = qkv_ps.tile([P, CHUNK], F32, tag="proj", name="proj")
            for k in range(NDT):
                nc.tensor.matmul(
                    ps[:, 0:DG],
                    xt[k][:, P * j:P * (j + 1)],
                    ws["wv"][:, k, :],
                    start=(k == 0), stop=(k == NDT - 1))
            dst = vaug[j][:].rearrange("p (h x) -> p h x", h=HEADS_PER_CORE)
            srcp = ps[:, 0:DG].rearrange("p (h x) -> p h x", h=HEADS_PER_CORE)
            # gpsimd cannot access PSUM; ACT has headroom after losing the
            # Wo-half/out-DMA work
            nc.scalar.activation(dst[:, :, 0:HD], srcp[:, :, :],
                                 mybir.ActivationFunctionType.Copy)
            nc.vector.tensor_copy(
                dst[:, :, HD:HD + 1],
                ones_r[:].rearrange("p (h x) -> p h x", x=1))

        # ---- Q^T / K^T projections (dg on partitions, fp16) -------------
        qt, kt = [], []
        for name, lst in (("wq", qt), ("wk", kt)):
            for m in range(NMT):
                lst.append(sbuf.tile([P, S], F16, tag=f"{name}T{m}",
                                     name=f"{name}T{m}"))

        def qk_unit(ci, m, name):
            lst = qt if name == "wq" else kt
            ps = qkv_ps.tile([P, CHUNK], F32, tag="proj", name="proj")
            for k in range(NDT):
                nc.tensor.matmul(
                    ps[:],
                    ws[name][:, k, P * m:P * (m + 1)],
                    xt[k][:, CHUNK * ci:CHUNK * (ci + 1)],
                    start=(k == 0), stop=(k == NDT - 1))
            dst = lst[m][:, CHUNK * ci:CHUNK * (ci + 1)]
            if biased:
                bname = "bq" if name == "wq" else "bk"
                # bias-add on ACT (only when biases are nonzero)
                nc.scalar.activation(
                    dst, ps[:],
                    mybir.ActivationFunctionType.Identity,
                    bias=biases[(bname, m)][:])
            else:
                nc.vector.tensor_copy(dst, ps[:])

        # ---- attention per (chunk, head pair) ---------------------------
        ctxT = [sbuf.tile([P, S], F16, tag=f"ctxT{m}", name=f"ctxT{m}")
                for m in range(NMT)]

        pending = []  # deferred per-(pair,hh) normalize closures

        def attention(ci, filler=()):
            """Emit chunk-ci attention; sprinkle `filler` unit closures
            (next-chunk projections / trailing Wo tiles) between j-steps so
            projection copy work never bunches up at chunk seams."""
            filler = list(filler)
            if "qkt" in ablate:
                for f in filler:
                    f()
                return
            jmax = 4 * ci + 3
            total_steps = NMT * (jmax + 1)
            step = 0
            emitted = 0

            def tick():
                nonlocal step, emitted
                step += 1
                # deferred divide ops first: they must be emitted before any
                # filler Wo unit that reads the ctx^T columns they write.
                # Consumption starts at tick 3 so the DVE reciprocal/multiply
                # never collide with the early diagonal masks (chunk 0 is
                # all-diagonal and needs the DVE at ticks 1-4).
                if pending and (step - 1) % (jmax + 1) >= 2:
                    pending.pop(0)()
                while emitted < len(filler) * step // total_steps:
                    filler[emitted]()
                    emitted += 1

            qsl = slice(CHUNK * ci, CHUNK * (ci + 1))
            for pair in range(NMT):
                pv = [pv_ps.tile([HD + 1, CHUNK], F32, tag="pv", name="pv")
                      for _ in range(2)]
                p2s = {}

                def nlo_of(j):
                    dd = j - 4 * ci
                    return P * dd if dd >= 0 else 0

                def emit_pv(j):
                    nlo = nlo_of(j)
                    p2 = p2s.pop(j)
                    if "pv" in ablate and j > 0:
                        return
                    for hh in range(2):
                        h = 2 * pair + hh
                        nc.tensor.matmul(
                            pv[hh][:, nlo:CHUNK],
                            vaug[j][:, (HD + 1) * h:(HD + 1) * (h + 1)],
                            p2[:, CHUNK * hh + nlo:CHUNK * (hh + 1)],
                            start=(j == 0), stop=(j == jmax),
                            skip_group_check=True)

                for j in range(jmax + 1):
                    nlo = nlo_of(j)
                    # QK^T: both heads as a concurrent row-tiled pair
                    st2 = stp_ps.tile([P, 2 * CHUNK], F32, tag="stp",
                                      name="stp")
                    for hh in range(2):
                        psl = slice(HD * hh, HD * (hh + 1))
                        nc.tensor.matmul(
                            st2[:, CHUNK * hh + nlo:CHUNK * (hh + 1)],
                            kt[pair][psl, P * j:P * (j + 1)],
                            qt[pair][psl, CHUNK * ci + nlo:CHUNK * (ci + 1)],
                            start=True, stop=True)
                    # exp over both heads in one ACT op (3D AP)
                    p2 = ptile_p.tile([P, 2 * CHUNK], F16, tag="p2",
                                      name="p2")
                    src = st2[:].rearrange("p (h q) -> p h q", h=2)
                    dst = p2[:].rearrange("p (h q) -> p h q", h=2)
                    if "exp" in ablate:
                        nc.vector.tensor_copy(dst[:, :, nlo:CHUNK],
                                              src[:, :, nlo:CHUNK])
                    else:
                        nc.scalar.activation(
                            dst[:, :, nlo:CHUNK], src[:, :, nlo:CHUNK],
                            mybir.ActivationFunctionType.Exp,
                            scale=0.125, bias=ebias[:])
                    # causal triangle mask on the diagonal block, split
                    # across DVE (hh0) and Pool (hh1) so neither strict-FIFO
                    # queue delays the dependent PV matmuls
                    if nlo > 0 or j == 4 * ci:
                        if "mask" not in ablate:
                            blk0 = slice(nlo, nlo + P)
                            nc.vector.tensor_mul(
                                p2[:, blk0], p2[:, blk0], m01[:])
                            blk1 = slice(CHUNK + nlo, CHUNK + nlo + P)
                            nc.gpsimd.affine_select(
                                out=p2[:, blk1], in_=p2[:, blk1],
                                compare_op=mybir.AluOpType.is_ge,
                                fill=0.0, base=0, pattern=[[1, P]],
                                channel_multiplier=-1)
                    p2s[j] = p2
                    if j - 4 >= 0:
                        emit_pv(j - 4)
                    tick()
                for j in (jmax - 3, jmax - 2, jmax - 1, jmax):
                    if j >= 0 and j in p2s:
                        emit_pv(j)

                # ---- softmax divide: ctx^T = pv / den -------------------
                # Deferred: runs 1-2 ticks into the next pair's j-loop so it
                # never delays the next pair's masks. Reads straight from
                # the PV psum (reciprocal of row 64, broadcast, multiply);
                # the multiply is the op that frees the psum bank, well
                # before the next pair's PV(0) at tick 4.
                if "div" in ablate:
                    for hh in range(2):
                        nc.vector.tensor_copy(
                            ctxT[pair][HD * hh:HD * (hh + 1), qsl],
                            pv[hh][0:HD, :])
                else:
                    for hh in range(2):
                        def norm(pv=pv, pair=pair, hh=hh, qsl=qsl):
                            rden = den_p.tile([1, CHUNK], F16,
                                              tag=f"rden{hh}",
                                              name=f"rden{hh}")
                            with nc.allow_low_precision(
                                    reason="1/den fits fp16: den in "
                                           "[e^-6, ~1e4], rel err 5e-4"):
                                nc.vector.reciprocal(rden[:],
                                                     pv[hh][HD:HD + 1, :])
                            rbc = den_p.tile([HD, CHUNK], F16,
                                             tag=f"rbc{hh}",
                                             name=f"rbc{hh}")
                            nc.gpsimd.partition_broadcast(rbc[0:HD, :],
                                                          rden[:])
                            nc.vector.tensor_mul(
                                ctxT[pair][HD * hh:HD * (hh + 1), qsl],
                                pv[hh][0:HD, :], rbc[0:HD, :])

                        pending.append(norm)

        # ---- Wo projection (fp16 out) -----------------------------------
        def wo_unit(i):
            ot = out_p.tile([P, D], F16, tag="ot", name="ot")
            pse = [qkv_ps.tile([P, CHUNK], F32, tag="proj", name="proj")
                   for _ in range(2)]
            for m in range(NMT):
                for e in range(2):
                    nc.tensor.matmul(
                        pse[e][:],
                        ctxT[m][:, P * i:P * (i + 1)],
                        wo[:, m, CHUNK * e:CHUNK * (e + 1)],
                        start=(m == 0), stop=(m == NMT - 1))
            # psum->sbuf copies both on DVE (gpsimd cannot access PSUM; ACT
            # stays dedicated to exp + V copies)
            nc.vector.tensor_copy(ot[:, 0:CHUNK], pse[0][:])
            nc.vector.tensor_copy(ot[:, CHUNK:D], pse[1][:])
            if "outdma" not in ablate:
                # ACT HWDGE, as in v1: SP would hold the next iteration's
                # input prefetch behind these (in-order queue), and gpsimd
                # SWDGE costs ~1us of Pool time per DMA (both measured
                # slower on HW by ~40us/iter)
                nc.scalar.dma_start(o_d.ap()[P * i:P * (i + 1), :], ot[:])

        def emit_compute():
            emit_consts()
            # prologue: chunk-0 projections
            for j in range(4):
                v_proj(j)
            for m in range(NMT):
                for name in ("wq", "wk"):
                    qk_unit(0, m, name)
            for ci in range(N_CH):
                if ci + 1 < N_CH:
                    cn = ci + 1
                    units = []
                    qs = [(m, name) for m in range(NMT)
                          for name in ("wq", "wk")]
                    for idx in range(4):
                        units.append(
                            lambda j=4 * cn + idx: v_proj(j))
                        m, name = qs[idx]
                        units.append(
                            lambda cn=cn, m=m, name=name:
                            qk_unit(cn, m, name))
                else:
                    # trailing chunk: interleave Wo tiles of chunks 0-2
                    units = [lambda i=i: wo_unit(i) for i in range(12)]
                attention(ci, units)
            while pending:  # last pair's normalizes, before the Wo tail
                pending.pop(0)()
            for i in range(12 if N_CH > 1 else 0, S // P):
                wo_unit(i)

        if "indma" in ablate and unroll > 1:
            emit_in_dma()
            with tc.For_i(0, unroll, 1):
                emit_compute()
        elif unroll > 1:
            with tc.For_i(0, unroll, 1):
                emit_in_dma()
                emit_compute()
        else:
            emit_in_dma()
            if warmup:
                emit_warmup()
            emit_compute()


def _shard_inputs(x, Wq, bq, Wk, bk, Wv, bv, Wo, bo, biased=False):
    x = np.asarray(x, np.float32)
    in_maps = []
    for core in range(N_CORES):
        b, g = divmod(core, 4)
        ds = slice(DG * g, DG * (g + 1))
        m = {
            "xT": np.ascontiguousarray(x[b].T).astype(np.float16),
            "wq": np.ascontiguousarray(
                np.asarray(Wq, np.float32)[:, ds]).astype(np.float16),
            "wk": np.ascontiguousarray(
                np.asarray(Wk, np.float32)[:, ds]).astype(np.float16),
            "wv": np.ascontiguousarray(
                np.asarray(Wv, np.float32)[:, ds]).astype(np.float16),
            "wo": np.ascontiguousarray(
                np.asarray(Wo, np.float32)[ds, :]).astype(np.float16),
        }
        if biased:
            m["bq"] = np.asarray(bq, np.float32)[ds].reshape(DG, 1).copy()
            m["bk"] = np.asarray(bk, np.float32)[ds].reshape(DG, 1).copy()
        in_maps.append(m)
    return in_maps


def kernel(x, Wq, bq, Wk, bk, Wv, bv, Wo, bo):
    mm_dt = _CACHE.get("mm_dt", "f16in")
    _CACHE["mm_dt"] = mm_dt
    biased = bool(np.any(np.asarray(bq)) or np.any(np.asarray(bk)))
    key = ("nc", biased)
    if key not in _CACHE:
        _CACHE[key] = build_kernel(mm_dt, biased=biased)
    nc = _CACHE[key]
    _CACHE["nc"] = nc  # test.py compatibility
    in_maps = _shard_inputs(x, Wq, bq, Wk, bk, Wv, bv, Wo, bo, biased)
    res = run_bass_kernel_spmd(
        nc, in_maps, core_ids=list(range(N_CORES)), trace=False)
    out = np.zeros((B, S, D), np.float32)
    for core in range(N_CORES):
        out[core // 4] += np.asarray(res.results[core]["o"], np.float32)
    # exact bias folding: +bo, + bv @ Wo (constant row vector)
    out += (np.asarray(bo, np.float32)
            + np.asarray(bv, np.float32) @ np.asarray(Wo, np.float32))
    return out


# revision 29
# speedup vs baseline: 1.5976x; 1.5976x over previous
"""Causal multi-head attention on 8 trn2 NeuronCores.

Problem: B=2, S=2048, D=1024, H=16 heads, HD=64. fp32 in/out.

Sharding: 8 cores = 2 (batch) x 4 (head groups of 4 heads).
Each core computes, for its batch b and head group g:
  Q^T,K^T [256, 2048] (dg on partitions, seq on free) = W^T-slice @ x
  V       [2048, 4*(64+1)]  (natural, a ones column per head)
  per 512-wide q chunk, per head-pair: for each k tile j:
    S^T[k,q] both heads as a ROW-TILED CONCURRENT matmul pair (K=64 each,
    tile_position rows 0-63 / 64-127, ~1.9x PE throughput measured) into
    one [128, 1024] PSUM tile;
    P = exp(S^T/8 - 4) on ACT (fp16 out; the -4 offset prevents fp16
    overflow and cancels exactly in the softmax ratio);
    causal: diagonal k-tiles narrowed to valid q columns, plus a [128,128]
    triangle mask on the diagonal block (split DVE/gpsimd, one per head,
    so neither strict-FIFO queue delays PV);
    PV accumulated over j with V_aug stationary (m=65; row 64 = softmax
    denominator), software-pipelined 4 j-steps behind QK so the ACT exp
    and mask latency never stall the PE FIFO.
  Normalize: one DVE copy frees the PV psum bank (shortest WAR chain for
  the next head-pair); reciprocal + gpsimd partition_broadcast + multiply
  into ctx^T (fp16) then run off the critical path on SBUF data.
  O_partial = ctx^T.T @ Wo_rows [2048, 1024] (fp16 out, ACT HWDGE queue).
Emission interleaves next-chunk projections (and trailing Wo tiles) into
the attention j-loops so projection ACT/DVE work never bunches up at
chunk seams. Engine balance: PE matmuls; ACT exp + QK bias-add + V
copies; DVE masks(h0)/normalize; gpsimd masks(h1)/broadcast.
Host: sums the 4 head-group partials per batch and adds bo + bv @ Wo.

All matmul operands fp16 (1 cycle/row PE rate, halved DMA + SBUF);
accumulation is always fp32 PSUM.
Measured (final): rel err 7.075e-4; 174-177 us/iter (pair-median 174.4,
iqr 167-182) vs the 264 us baseline — ~1.5x. Engine-occupancy model (TimelineSim): PE 114 us
busy, ACT 92, DVE 46, Pool 31; the HW-vs-sim gap is ~70 ns/matmul
issue+LDWEIGHTS overhead (measured via microbenchmarks) plus the For_i
back-edge all-engine barrier + input-DMA refill (~13 us/iter).
"""

import sys

if "/opt/trn_rl_repo" not in sys.path:
    sys.path.insert(0, "/opt/trn_rl_repo")

import numpy as np

import concourse.bacc as bacc
import concourse.bass as bass
import concourse.mybir as mybir
import concourse.tile as tile
from concourse.bass_utils import run_bass_kernel_spmd

B, S, D, H = 2, 2048, 1024, 16
HD = D // H  # 64
N_CORES = 8
HEADS_PER_CORE = H // 4  # 4
DG = HEADS_PER_CORE * HD  # 256 head dims per core
P = 128
CHUNK = 512  # q chunk width
N_KT = S // P  # 16 k tiles
N_CH = S // CHUNK  # 4 q chunks
F32 = mybir.dt.float32
F16 = mybir.dt.float16
EXP_BIAS = -4.0  # exp(s/8 - 4): fp16-overflow guard, cancels in softmax

_CACHE = {}


def build_kernel(mm_dt="f16in", unroll=1, ablate=(), biased=False,
                 warmup=True, xt_rows=False):
    """Build + compile the per-core SPMD program. unroll>1 wraps the body
    in a hardware loop (for pure device timing measurements)."""
    nc = bacc.Bacc("TRN2", target_bir_lowering=False, debug=False)
    xT_d = nc.dram_tensor("xT", [D, S], F16, kind="ExternalInput")
    wq_d = nc.dram_tensor("wq", [D, DG], F16, kind="ExternalInput")
    wk_d = nc.dram_tensor("wk", [D, DG], F16, kind="ExternalInput")
    wv_d = nc.dram_tensor("wv", [D, DG], F16, kind="ExternalInput")
    wo_d = nc.dram_tensor("wo", [DG, D], F16, kind="ExternalInput")
    if biased:
        bq_d = nc.dram_tensor("bq", [DG, 1], F32, kind="ExternalInput")
        bk_d = nc.dram_tensor("bk", [DG, 1], F32, kind="ExternalInput")
    else:
        bq_d = bk_d = None
    o_d = nc.dram_tensor("o", [S, D], F16, kind="ExternalOutput")

    NDT = D // P  # 8 contraction tiles over D
    NMT = DG // P  # 2 m-tiles over the core's head dims (= head pairs)

    with tile.TileContext(nc) as tc:
        _body(tc, nc,
              xT_d, wq_d, wk_d, wv_d, wo_d, bq_d, bk_d, o_d, NDT, NMT,
              ablate, unroll, warmup, xt_rows)

    nc.compile()
    return nc


def _body(tc, nc, xT_d, wq_d, wk_d, wv_d, wo_d, bq_d, bk_d, o_d,
          NDT, NMT, ablate=(), unroll=1, warmup=True, xt_rows=False):
    import contextlib
    ctx = contextlib.ExitStack()
    biased = bq_d is not None
    XT_ROWS = xt_rows
    LAG = 4  # j-steps the PV matmuls trail the QK/exp pipeline
    for a in ablate:
        if a.startswith("lag"):
            LAG = int(a[3:])
    with ctx:
        const = ctx.enter_context(tc.tile_pool(name="const", bufs=1))
        sbuf = ctx.enter_context(tc.tile_pool(name="sbuf", bufs=1))
        ptile_p = ctx.enter_context(tc.tile_pool(name="ptile", bufs=8))
        den_p = ctx.enter_context(tc.tile_pool(name="den", bufs=6))
        ctxu_p = ctx.enter_context(tc.tile_pool(name="ctxu", bufs=6))
        out_p = ctx.enter_context(tc.tile_pool(name="outp", bufs=3))
        qkv_ps = ctx.enter_context(
            tc.tile_pool(name="qkv_ps", bufs=2, space="PSUM"))
        stp_ps = ctx.enter_context(
            tc.tile_pool(name="stp_ps", bufs=2, space="PSUM"))
        pv_ps = ctx.enter_context(
            tc.tile_pool(name="pv_ps", bufs=2, space="PSUM"))

        # ---- input tiles ------------------------------------------------
        xt = [const.tile([P, S], F16, tag=f"xt{i}", name=f"xt{i}")
              for i in range(NDT)]
        # weights as single [128, k-tile, dg] tiles -> one DMA config each
        wst = {name: const.tile([P, NDT, DG], F16, tag=name, name=name)
               for name in ("wq", "wk", "wv")}
        ws = {name: [wst[name][:, i, :] for i in range(NDT)]
              for name in ("wq", "wk", "wv")}
        wot = const.tile([P, NMT, D], F16, tag="wo", name="wo")
        wo = [wot[:, m, :] for m in range(NMT)]
        if biased:
            biases = {(name, m): const.tile([P, 1], F32, tag=f"{name}{m}",
                                            name=f"{name}{m}")
                      for name in ("bq", "bk") for m in range(NMT)}

        def dma_xt(ci):
            if XT_ROWS:
                if ci != 0:
                    return
                for k in range(NDT):
                    nc.sync.dma_start(xt[k][:],
                                      xT_d.ap()[P * k:P * (k + 1), :])
                return
            csl = slice(CHUNK * ci, CHUNK * (ci + 1))
            for k in range(NDT):
                nc.sync.dma_start(xt[k][:, csl],
                                  xT_d.ap()[P * k:P * (k + 1), csl])

        def emit_in_dma():
            # order: V(0)+QK(0) deps first, then remaining chunks, wo last.
            # Weights are 1 config each on the SP queue, so config
            # serialization (565 ns each) never gates compute in a cold exec.
            nc.sync.dma_start(
                wst["wv"][:], wv_d.ap().rearrange("(k p) d -> p k d", p=P))
            dma_xt(0)
            nc.sync.dma_start(
                wst["wq"][:], wq_d.ap().rearrange("(k p) d -> p k d", p=P))
            nc.sync.dma_start(
                wst["wk"][:], wk_d.ap().rearrange("(k p) d -> p k d", p=P))
            if biased:
                for (name, m), t in biases.items():
                    d = bq_d if name == "bq" else bk_d
                    nc.sync.dma_start(t[:], d.ap()[P * m:P * (m + 1), :])
            for ci in range(1, N_CH):
                dma_xt(ci)
            nc.sync.dma_start(
                wot[:], wo_d.ap().rearrange("(m p) d -> p m d", p=P))

        # ---- constants: vaug ones + causal triangle mask ----------------
        ones_r = const.tile([P, HEADS_PER_CORE], F16, tag="ones_r",
                            name="ones_r")
        ebias = const.tile([P, 1], F32, tag="ebias", name="ebias")
        m01 = const.tile([P, P], F16, tag="m01", name="m01")
        wup = const.tile([P, CHUNK], F16, tag="wup", name="wup")

        def emit_warmup(n=16):
            # dummy matmuls with no DMA dependency: the PE p-state ramp
            # (0.65/1.2 GHz until ~3us of continuous busy) completes during
            # the input-DMA prologue instead of on the first real matmuls
            nc.vector.memset(wup[:], 0.0)
            ps = qkv_ps.tile([P, CHUNK], F32, tag="proj", name="proj")
            for _ in range(n):
                nc.tensor.matmul(ps[:], wup[:, 0:P], wup[:],
                                 start=True, stop=True)

        # ---- V projection (natural layout + ones cols) ------------------
        # vaug[j]: [128, 4*65]; head h cols h*65..h*65+63 = V, col h*65+64 = 1
        vaug = [sbuf.tile([P, HEADS_PER_CORE * (HD + 1)], F16,
                          tag=f"vaug{j}", name=f"vaug{j}")
                for j in range(N_KT)]

        def emit_consts():
            # emitted once, before the (possibly unrolled) compute body:
            # all of these are idempotent across iterations, including the
            # vaug ones-columns (v_proj only ever rewrites cols 0:HD)
            nc.vector.memset(ones_r[:], 1.0)
            nc.vector.memset(ebias[:], EXP_BIAS)
            # m01[r, c] = 1 if c >= r else 0 (causal triangle, q >= key)
            nc.gpsimd.memset(m01[:], 1.0)
            nc.gpsimd.affine_select(
                out=m01[:], in_=m01[:],
                compare_op=mybir.AluOpType.is_ge,
                fill=0.0, base=0, pattern=[[1, P]],
                channel_multiplier=-1)
            for j in range(N_KT):
                dst = vaug[j][:].rearrange("p (h x) -> p h x",
                                           h=HEADS_PER_CORE)
                nc.vector.tensor_copy(
                    dst[:, :, HD:HD + 1],
                    ones_r[:].rearrange("p (h x) -> p h x", x=1))

        def v_proj(j):
            ps = qkv_ps.tile([P, CHUNK], F32, tag="proj", name="proj")
            for k in range(NDT):
                nc.tensor.matmul(
                    ps[:, 0:DG],
                    xt[k][:, P * j:P * (j + 1)],
                    ws["wv"][k][:],
                    start=(k == 0), stop=(k == NDT - 1))
            dst = vaug[j][:].rearrange("p (h x) -> p h x", h=HEADS_PER_CORE)
            srcp = ps[:, 0:DG].rearrange("p (h x) -> p h x", h=HEADS_PER_CORE)
            # ACT copy keeps the (busy, strictly-FIFO) DVE off the PV
            # dependency chain
            nc.scalar.activation(dst[:, :, 0:HD], srcp[:, :, :],
                                 mybir.ActivationFunctionType.Copy)

        # ---- Q^T / K^T projections (dg on partitions, fp16) -------------
        qt, kt = [], []
        for name, lst in (("wq", qt), ("wk", kt)):
            for m in range(NMT):
                lst.append(sbuf.tile([P, S], F16, tag=f"{name}T{m}",
                                     name=f"{name}T{m}"))

        def qk_unit(ci, m, name):
            lst = qt if name == "wq" else kt
            ps = qkv_ps.tile([P, CHUNK], F32, tag="proj", name="proj")
            for k in range(NDT):
                nc.tensor.matmul(
                    ps[:],
                    ws[name][k][:, P * m:P * (m + 1)],
                    xt[k][:, CHUNK * ci:CHUNK * (ci + 1)],
                    start=(k == 0), stop=(k == NDT - 1))
            # copy (+bias when nonzero) on ACT: keeps DVE out of the QK^T
            # dep chain
            if biased:
                bname = "bq" if name == "wq" else "bk"
                nc.scalar.activation(
                    lst[m][:, CHUNK * ci:CHUNK * (ci + 1)], ps[:],
                    mybir.ActivationFunctionType.Identity,
                    bias=biases[(bname, m)][:])
            else:
                nc.scalar.activation(
                    lst[m][:, CHUNK * ci:CHUNK * (ci + 1)], ps[:],
                    mybir.ActivationFunctionType.Copy)

        # ---- attention per (chunk, head pair) ---------------------------
        ctxT = [sbuf.tile([P, S], F16, tag=f"ctxT{m}", name=f"ctxT{m}")
                for m in range(NMT)]

        pending = []  # deferred per-(pair,hh) normalize closures

        def attention(ci, filler=()):
            """Emit chunk-ci attention; sprinkle `filler` unit closures
            (next-chunk projections / trailing Wo tiles) between j-steps so
            projection ACT/DVE work never bunches up at chunk seams."""
            filler = list(filler)
            if "qkt" in ablate:
                for f in filler:
                    f()
                return
            jmax = 4 * ci + 3
            total_steps = NMT * (jmax + 1)
            step = 0
            emitted = 0
            if "fullphase" in ablate and ci >= 1:
                # emit fillers only on full-width (non-diagonal) j-steps:
                # their DVE/ACT copy work never lands in the FIFO ahead of
                # the diagonal-phase masks that gate PV
                allowed = set()
                for pr in range(NMT):
                    for j in range(4 * ci):
                        allowed.add(pr * (jmax + 1) + j + 1)
            else:
                allowed = None
            n_allowed = len(allowed) if allowed is not None else total_steps

            def tick():
                nonlocal step, emitted
                step += 1
                # deferred divide ops first: they must be emitted before any
                # filler Wo unit that reads the ctx^T columns they write.
                # Consumption starts at tick 3 so the DVE reciprocal/multiply
                # never collide with the early diagonal masks (chunk 0 is
                # all-diagonal and needs the DVE at ticks 1-4).
                if pending and (step - 1) % (jmax + 1) >= 2:
                    pending.pop(0)()
                if allowed is None:
                    share = step
                elif step in allowed:
                    share = sorted(allowed).index(step) + 1
                else:
                    return
                while emitted < len(filler) * share // n_allowed:
                    filler[emitted]()
                    emitted += 1

            qsl = slice(CHUNK * ci, CHUNK * (ci + 1))
            for pair in range(NMT):
                pv = [pv_ps.tile([HD + 1, CHUNK], F32, tag="pv", name="pv")
                      for _ in range(2)]
                p2s = {}

                def nlo_of(j):
                    dd = j - 4 * ci
                    return P * dd if dd >= 0 else 0

                def emit_pv(j):
                    nlo = nlo_of(j)
                    p2 = p2s.pop(j)
                    if "pv" in ablate and j > 0:
                        return
                    for hh in range(2):
                        h = 2 * pair + hh
                        nc.tensor.matmul(
                            pv[hh][:, nlo:CHUNK],
                            vaug[j][:, (HD + 1) * h:(HD + 1) * (h + 1)],
                            p2[:, CHUNK * hh + nlo:CHUNK * (hh + 1)],
                            start=(j == 0), stop=(j == jmax),
                            skip_group_check=True)

                for j in range(jmax + 1):
                    nlo = nlo_of(j)
                    w = CHUNK - nlo
                    # QK^T: both heads as a concurrent row-tiled pair
                    st2 = stp_ps.tile([P, 2 * CHUNK], F32, tag="stp",
                                      name="stp")
                    for hh in range(2):
                        psl = slice(HD * hh, HD * (hh + 1))
                        nc.tensor.matmul(
                            st2[:, CHUNK * hh + nlo:CHUNK * (hh + 1)],
                            kt[pair][psl, P * j:P * (j + 1)],
                            qt[pair][psl, CHUNK * ci + nlo:CHUNK * (ci + 1)],
                            start=True, stop=True)
                    # exp over both heads in one ACT op (3D AP)
                    p2 = ptile_p.tile([P, 2 * CHUNK], F16, tag="p2",
                                      name="p2")
                    src = st2[:].rearrange("p (h q) -> p h q", h=2)
                    dst = p2[:].rearrange("p (h q) -> p h q", h=2)
                    if "exp" in ablate:
                        nc.vector.tensor_copy(dst[:, :, nlo:CHUNK],
                                              src[:, :, nlo:CHUNK])
                    else:
                        nc.scalar.activation(
                            dst[:, :, nlo:CHUNK], src[:, :, nlo:CHUNK],
                            mybir.ActivationFunctionType.Exp,
                            scale=0.125, bias=ebias[:])
                    # causal triangle mask on the diagonal block, split
                    # across DVE (hh0) and Pool (hh1) so neither strict-FIFO
                    # queue delays the dependent PV matmuls
                    if nlo > 0 or j == 4 * ci:
                        if "mask" not in ablate:
                            blk0 = slice(nlo, nlo + P)
                            nc.vector.tensor_mul(
                                p2[:, blk0], p2[:, blk0], m01[:])
                            blk1 = slice(CHUNK + nlo, CHUNK + nlo + P)
                            nc.gpsimd.affine_select(
                                out=p2[:, blk1], in_=p2[:, blk1],
                                compare_op=mybir.AluOpType.is_ge,
                                fill=0.0, base=0, pattern=[[1, P]],
                                channel_multiplier=-1)
                    p2s[j] = p2
                    if j - LAG >= 0:
                        emit_pv(j - LAG)
                    tick()
                for j in range(max(0, jmax - LAG + 1), jmax + 1):
                    if j in p2s:
                        emit_pv(j)

                # ---- softmax divide: ctx^T = pv / den -------------------
                # One plain copy frees the PV psum bank (shortest possible
                # WAR chain for the next pair); the reciprocal/broadcast/
                # multiply then run on SBUF data off the critical path.
                if "div" in ablate:
                    for hh in range(2):
                        nc.vector.tensor_copy(
                            ctxT[pair][HD * hh:HD * (hh + 1), qsl],
                            pv[hh][0:HD, :])
                elif "normdirect" in ablate:
                    # no cu staging: reciprocal/multiply read the PV psum;
                    # the deferred multiply is what frees the bank (still
                    # 1-2 ticks before the next pair's PV(0) at tick 4)
                    for hh in range(2):
                        def norm(pv=pv, pair=pair, hh=hh, qsl=qsl):
                            rden = den_p.tile([1, CHUNK], F16,
                                              tag=f"rden{hh}",
                                              name=f"rden{hh}")
                            with nc.allow_low_precision(
                                    reason="1/den fits fp16: den in "
                                           "[e^-6, ~1e4], rel err 5e-4"):
                                nc.vector.reciprocal(rden[:],
                                                     pv[hh][HD:HD + 1, :])
                            rbc = den_p.tile([HD, CHUNK], F16,
                                             tag=f"rbc{hh}",
                                             name=f"rbc{hh}")
                            nc.gpsimd.partition_broadcast(rbc[0:HD, :],
                                                          rden[:])
                            nc.vector.tensor_mul(
                                ctxT[pair][HD * hh:HD * (hh + 1), qsl],
                                pv[hh][0:HD, :], rbc[0:HD, :])

                        pending.append(norm)
                else:
                    for hh in range(2):
                        cu = ctxu_p.tile([HD + 1, CHUNK], F32,
                                         tag=f"cu{hh}", name=f"cu{hh}")
                        nc.vector.tensor_copy(cu[:], pv[hh][:])

                        def norm(cu=cu, pair=pair, hh=hh, qsl=qsl):
                            # deferred: runs 1-2 ticks into the next pair's
                            # j-loop, when the DVE/Pool queues are drained,
                            # so it never delays the next pair's masks
                            rden = den_p.tile([1, CHUNK], F16,
                                              tag=f"rden{hh}",
                                              name=f"rden{hh}")
                            with nc.allow_low_precision(
                                    reason="1/den fits fp16: den in "
                                           "[e^-6, ~1e4], rel err 5e-4"):
                                nc.vector.reciprocal(rden[:],
                                                     cu[HD:HD + 1, :])
                            rbc = den_p.tile([HD, CHUNK], F16,
                                             tag=f"rbc{hh}",
                                             name=f"rbc{hh}")
                            nc.gpsimd.partition_broadcast(rbc[0:HD, :],
                                                          rden[:])
                            nc.vector.tensor_mul(
                                ctxT[pair][HD * hh:HD * (hh + 1), qsl],
                                cu[0:HD, :], rbc[0:HD, :])

                        pending.append(norm)

        # ---- Wo projection (fp16 out) -----------------------------------
        obatch = {}

        def wo_unit(i):
            if "obatch" in ablate:
                if i % 2 == 0:
                    obatch["ot"] = out_p.tile([P, 2, D], F16, tag="ot",
                                              name="ot")
                ot = obatch["ot"][:, i % 2, :]
            else:
                ot = out_p.tile([P, D], F16, tag="ot", name="ot")[:]
            pse = [qkv_ps.tile([P, CHUNK], F32, tag="proj", name="proj")
                   for _ in range(2)]
            for m in range(NMT):
                for e in range(2):
                    nc.tensor.matmul(
                        pse[e][:],
                        ctxT[m][:, P * i:P * (i + 1)],
                        wo[m][:, CHUNK * e:CHUNK * (e + 1)],
                        start=(m == 0), stop=(m == NMT - 1))
            for e in range(2):
                # ACT copy by default: keeps the DVE queue free for the
                # causal masks that gate PV in the diagonal-heavy trailing
                # chunk ("wodve" moves both to DVE; pair with "fullphase"
                # so they only ever land on full-width ticks)
                esl = ot[:, CHUNK * e:CHUNK * (e + 1)]
                if "wodve" in ablate or (e == 1 and "woe1dve" in ablate):
                    nc.vector.tensor_copy(esl, pse[e][:])
                else:
                    nc.scalar.activation(esl, pse[e][:],
                                         mybir.ActivationFunctionType.Copy)
            if "outdma" not in ablate:
                # output rides the ACT HWDGE queue: the SP input queue is
                # in-order, so putting outputs there would block the next
                # loop iteration's input prefetch behind this iteration's
                # compute
                if "obatch" in ablate:
                    if i % 2 == 1:
                        nc.scalar.dma_start(
                            o_d.ap()[P * (i - 1):P * (i + 1), :].rearrange(
                                "(h p) d -> p h d", p=P),
                            obatch["ot"][:])
                else:
                    nc.scalar.dma_start(o_d.ap()[P * i:P * (i + 1), :], ot)

        def emit_compute():
            # prologue: chunk-0 projections
            for j in range(4):
                v_proj(j)
            for m in range(NMT):
                for name in ("wq", "wk"):
                    qk_unit(0, m, name)
            for ci in range(N_CH):
                if ci + 1 < N_CH:
                    cn = ci + 1
                    units = []
                    qs = [(m, name) for m in range(NMT)
                          for name in ("wq", "wk")]
                    for idx in range(4):
                        units.append(
                            lambda j=4 * cn + idx: v_proj(j))
                        m, name = qs[idx]
                        units.append(
                            lambda cn=cn, m=m, name=name:
                            qk_unit(cn, m, name))
                else:
                    # trailing chunk: interleave Wo tiles of chunks 0-2
                    units = [lambda i=i: wo_unit(i) for i in range(12)]
                attention(ci, units)
            while pending:  # last pair's normalizes, before the Wo tail
                pending.pop(0)()
            for i in range(12 if N_CH > 1 else 0, S // P):
                wo_unit(i)

        if "indma" in ablate and unroll > 1:
            emit_in_dma()
            emit_consts()
            with tc.For_i(0, unroll, 1):
                emit_compute()
        elif unroll > 1:
            emit_consts()
            with tc.For_i(0, unroll, 1):
                emit_in_dma()
                emit_compute()
        else:
            emit_in_dma()
            emit_consts()
            if warmup:
                emit_warmup()
            emit_compute()


def _shard_inputs(x, Wq, bq, Wk, bk, Wv, bv, Wo, bo, biased=False):
    x = np.asarray(x, np.float32)
    in_maps = []
    for core in range(N_CORES):
        b, g = divmod(core, 4)
        ds = slice(DG * g, DG * (g + 1))
        m = {
            "xT": np.ascontiguousarray(x[b].T).astype(np.float16),
            "wq": np.ascontiguousarray(
                np.asarray(Wq, np.float32)[:, ds]).astype(np.float16),
            "wk": np.ascontiguousarray(
                np.asarray(Wk, np.float32)[:, ds]).astype(np.float16),
            "wv": np.ascontiguousarray(
                np.asarray(Wv, np.float32)[:, ds]).astype(np.float16),
            "wo": np.ascontiguousarray(
                np.asarray(Wo, np.float32)[ds, :]).astype(np.float16),
        }
        if biased:
            m["bq"] = np.asarray(bq, np.float32)[ds].reshape(DG, 1).copy()
            m["bk"] = np.asarray(bk, np.float32)[ds].reshape(DG, 1).copy()
        in_maps.append(m)
    return in_maps


def kernel(x, Wq, bq, Wk, bk, Wv, bv, Wo, bo):
    mm_dt = _CACHE.get("mm_dt", "f16in")
    _CACHE["mm_dt"] = mm_dt
    biased = bool(np.any(np.asarray(bq)) or np.any(np.asarray(bk)))
    key = ("nc", biased)
    if key not in _CACHE:
        _CACHE[key] = build_kernel(mm_dt, biased=biased)
    _CACHE["nc"] = _CACHE[key]
    nc = _CACHE[key]
    in_maps = _shard_inputs(x, Wq, bq, Wk, bk, Wv, bv, Wo, bo, biased)
    res = run_bass_kernel_spmd(
        nc, in_maps, core_ids=list(range(N_CORES)), trace=False)
    out = np.zeros((B, S, D), np.float32)
    for core in range(N_CORES):
        out[core // 4] += np.asarray(res.results[core]["o"], np.float32)
    # exact bias folding: +bo, + bv @ Wo (constant row vector)
    out += (np.asarray(bo, np.float32)
            + np.asarray(bv, np.float32) @ np.asarray(Wo, np.float32))
    return out

